# revision 3
# baseline (speedup 1.0000x reference)
"""Bass/Trainium2 kernel for nn_AttentionHead (B=4, C=D=64, H=W=64).

Sharding: 8 cores = 4 batches x 2 query-halves. Each core holds the full
x for its batch (keys/values need all 4096 positions) and computes
attention for 2048 query positions. Per-core inputs are column-rotated so
every core's query block is columns [0, 2048) of its own x — the program
is identical across cores (SPMD), only data differs. Softmax over keys is
permutation-invariant, so rotating the key order is exact.

On-device math (per core, N=4096 keys, NH=2048 queries):
  GroupNorm(num_groups=C) is affine per (batch, channel): xn = s*x + t with
  s = gn_w*rsqrt(var+eps), t = gn_b - mu*s. Folded into the projections:
  Q = (wq*s) @ x + (wq@t + bq), etc. An all-ones row appended to x makes
  every projection bias a plain matmul contraction (x_aug is [65, 4096]).
  Scores are computed transposed, S^T[m,n] = K[:,m]^T Q[:,n], so softmax'
  numerator exp(S^T/8) lands with keys on partitions — ready to be the
  moving operand of the AV matmul. No max-subtraction (|s/8| < ~2.5 here,
  exp is safe in fp32). The softmax denominator comes from an extra
  all-ones column in V' (so O'[64,n] = sum_m exp), and the final division
  is folded past the output projection: y = (wo@O + bo*denom) * (1/denom),
  with bo riding as row 64 of the augmented wo^T.
"""

import sys

sys.path.insert(0, "/opt/trn_rl_repo")

from contextlib import ExitStack

import numpy as np

import concourse.bass as bass
import concourse.tile as tile
from concourse import mybir
from concourse.bass import ts
from concourse.bass_utils import run_bass_kernel_spmd

# ---------------------------------------------------------------------------
# Workaround: this toolchain's walrus accepts at most ONE semaphore wait per
# instruction, but Tile's scheduler (and its tail drain) can attach several.
# Post-process every block, hoisting excess waits onto InstEventSemaphore
# carriers inserted immediately before the offending instruction on the same
# engine (engines execute their streams in order => semantically identical).
from concourse.vector_clock import ScopedClock as _ScopedClock
from concourse.bass import _bass_rust as _br


def _split_multiwait_instructions(nc, h0):
    cur_bb = nc.cur_bb.bb
    for f in nc.m.functions:
        for bb in f.blocks:
            insts = list(bb.instructions)
            out = []
            changed = False
            for ins in insts:
                si = ins.sync_info
                if si is not None:
                    waits = list(si.on_wait)
                    if len(waits) > 1:
                        for w in waits[:-1]:
                            carrier = nc.engines[ins.engine].wait_ge(h0, 0).ins
                            lst = list(cur_bb.instructions)
                            assert lst and lst[-1].name == carrier.name
                            lst.pop()
                            cur_bb.instructions = lst
                            carrier.sync_info.on_wait = [w]
                            out.append(carrier)
                        si.on_wait = [waits[-1]]
                        changed = True
                out.append(ins)
            if changed:
                bb.instructions = out


def _patched_drain_and_barrier(self, tick_clock, wait_clock):
    nc = self.nc
    assert self.sems is not None
    h0 = next(iter(self.sems.allocated().values()), None)
    if h0 is not None:
        _split_multiwait_instructions(nc, h0)

    drain_inst = nc.sync.drain()
    wait_clock.add_sem_waits(
        drain_inst.ins, _ScopedClock({None: tick_clock.global_clock})
    )
    si = drain_inst.ins.sync_info
    if si is not None:
        waits = list(si.on_wait)
        if len(waits) > 1:
            si.on_wait = [waits[0]]
            for w in waits[1:]:
                d2 = nc.sync.drain()
                _br.wait_op(d2.ins, h0, 0, "sem-ge", False)
                d2.ins.sync_info.on_wait = [w]

    nc.all_engine_barrier()
    popped = nc._tile_sem_poison_stack.pop()
    assert popped is self._sem_poison
    nc.clear_and_free_semaphores(list(self.sems.allocated().values()))
    nc.all_engine_barrier()


tile.TileContext._drain_and_barrier = _patched_drain_and_barrier
# ---------------------------------------------------------------------------

B, C, D, H, W = 4, 64, 64, 64, 64
N = H * W  # 4096 spatial positions (keys)
NCORES = 8
NH = N // 2  # 2048 queries per core
NT = 512  # query-tile width (one PSUM bank)
MT = 128  # key-tile height (matmul partition dim)
NJ = NH // NT  # 4 query tiles
NM = N // MT  # 32 key tiles
GRP = 3  # key tiles per scores/exp group (3 PSUM banks)
EPS = 1e-5
SCALE = 1.0 / np.sqrt(np.float32(D))  # folded into exp's free affine
F32 = mybir.dt.float32

_cache = {}


def _build_nc():
    nc = bass.Bass()
    x_d = nc.declare_dram_parameter("x", [C, N], F32, isOutput=False)
    wqT_d = nc.declare_dram_parameter("wqT", [C, D], F32, isOutput=False)
    wkT_d = nc.declare_dram_parameter("wkT", [C, D], F32, isOutput=False)
    wvT_d = nc.declare_dram_parameter("wvT", [C, D], F32, isOutput=False)
    woTa_d = nc.declare_dram_parameter("woTa", [D + 1, D], F32, isOutput=False)
    bq_d = nc.declare_dram_parameter("bq_row", [1, D], F32, isOutput=False)
    bk_d = nc.declare_dram_parameter("bk_row", [1, D], F32, isOutput=False)
    bv_d = nc.declare_dram_parameter("bv_row", [1, D], F32, isOutput=False)
    gnw_d = nc.declare_dram_parameter("gnw", [C, 1], F32, isOutput=False)
    gnb_d = nc.declare_dram_parameter("gnb", [C, 1], F32, isOutput=False)
    out_d = nc.declare_dram_parameter("out", [D, NH], F32, isOutput=True)

    with tile.TileContext(nc) as tc, ExitStack() as ctx:
        consts = ctx.enter_context(tc.tile_pool(name="consts", bufs=1))
        big = ctx.enter_context(tc.tile_pool(name="big", bufs=1))
        exps = ctx.enter_context(tc.tile_pool(name="exps", bufs=2))
        outp = ctx.enter_context(tc.tile_pool(name="outp", bufs=2))
        ps_s = ctx.enter_context(tc.tile_pool(name="ps_s", bufs=2, space="PSUM"))
        ps_o = ctx.enter_context(tc.tile_pool(name="ps_o", bufs=1, space="PSUM"))
        ps_m = ctx.enter_context(tc.tile_pool(name="ps_m", bufs=1, space="PSUM"))

        # ---- load weights / params -------------------------------------
        wqT = consts.tile([C, D], F32)
        wkT = consts.tile([C, D], F32)
        wvT = consts.tile([C, D], F32)
        woTa = consts.tile([D + 1, D], F32)
        bq_row = consts.tile([1, D], F32)
        bk_row = consts.tile([1, D], F32)
        bv_row = consts.tile([1, D], F32)
        gnw = consts.tile([C, 1], F32)
        gnb = consts.tile([C, 1], F32)
        dma = nc.default_dma_engine
        dma.dma_start(out=wqT, in_=wqT_d[:])
        dma.dma_start(out=wkT, in_=wkT_d[:])
        dma.dma_start(out=wvT, in_=wvT_d[:])
        dma.dma_start(out=woTa, in_=woTa_d[:])
        dma.dma_start(out=bq_row, in_=bq_d[:])
        dma.dma_start(out=bk_row, in_=bk_d[:])
        dma.dma_start(out=bv_row, in_=bv_d[:])
        dma.dma_start(out=gnw, in_=gnw_d[:])
        dma.dma_start(out=gnb, in_=gnb_d[:])

        ones_col = consts.tile([1, D], F32)
        nc.gpsimd.memset(ones_col, 1.0)

        # ---- x with an all-ones row 64 ---------------------------------
        x_aug = big.tile([C + 1, N], F32)
        dma.dma_start(out=x_aug[0:C, :], in_=x_d[:])
        nc.gpsimd.memset(x_aug[C : C + 1, :], 1.0)

        # ---- GroupNorm stats: mean/var per channel over all 4096 -------
        stats = consts.tile([C, N // 512, 6], F32)
        for j in range(N // 512):
            nc.vector.bn_stats(out=stats[:, j, :], in_=x_aug[0:C, ts(j, 512)])
        mv = consts.tile([C, 2], F32)
        nc.vector.bn_aggr(out=mv, in_=stats)

        # rs = (var+eps)^-0.5 via Ln/Exp (keeps everything in one ACT
        # table set, natural_log_exp_and_others, shared with the main exp)
        vpe = consts.tile([C, 1], F32)
        epst = consts.tile([C, 1], F32)
        nc.gpsimd.memset(epst, EPS)
        nc.vector.tensor_add(out=vpe, in0=mv[:, 1:2], in1=epst)
        lnv = consts.tile([C, 1], F32)
        nc.scalar.activation(out=lnv, in_=vpe, func=mybir.ActivationFunctionType.Ln)
        rs = consts.tile([C, 1], F32)
        nc.scalar.activation(
            out=rs, in_=lnv, func=mybir.ActivationFunctionType.Exp, scale=-0.5
        )
        s_vec = consts.tile([C, 1], F32)
        nc.vector.tensor_mul(out=s_vec, in0=rs, in1=gnw)
        mus = consts.tile([C, 1], F32)
        nc.vector.tensor_mul(out=mus, in0=mv[:, 0:1], in1=s_vec)
        t_vec = consts.tile([C, 1], F32)
        nc.vector.tensor_sub(out=t_vec, in0=gnb, in1=mus)

        # ---- augmented projection weights ------------------------------
        # what_* rows 0..63 = w^T * s (per-channel), row 64 = (w@t + b)^T
        what_q = consts.tile([C + 1, D], F32)
        what_k = consts.tile([C + 1, D], F32)
        what_v = consts.tile([C + 1, D + 1], F32)
        nc.gpsimd.memset(what_v[:, D : D + 1], 0.0)
        nc.vector.tensor_scalar_mul(out=what_q[0:C, :], in0=wqT, scalar1=s_vec)
        nc.vector.tensor_scalar_mul(out=what_k[0:C, :], in0=wkT, scalar1=s_vec)
        nc.vector.tensor_scalar_mul(out=what_v[0:C, 0:D], in0=wvT, scalar1=s_vec)
        for whT, wT, b_row, col in (
            (what_q, wqT, bq_row, D),
            (what_k, wkT, bk_row, D),
            (what_v, wvT, bv_row, D),
        ):
            r_ps = ps_m.tile([MT, NT], F32, tag="m")
            nc.tensor.matmul(r_ps[0:1, 0:D], t_vec, wT, start=True, stop=True)
            nc.vector.tensor_add(
                out=whT[C : C + 1, 0:col], in0=r_ps[0:1, 0:D], in1=b_row
            )
        nc.gpsimd.memset(what_v[C : C + 1, D : D + 1], 1.0)

        # ---- projections ----------------------------------------------
        # K [64, 4096] and Q [64, 2048] channel-major; V'^T as 32 tiles of
        # [128 keys, 65] (64 channels + ones column), packed contiguously.
        k_sb = big.tile([D, N], F32)
        q_sb = big.tile([D, NH], F32)
        vt_sb = big.tile([MT, NM * (D + 1)], F32)

        for grp in ((0, 1, 2), (3, 4, 5), (6, 7)):
            p = ps_s.tile([MT, GRP * NT], F32, tag="sps")
            for k, j in enumerate(grp):
                nc.tensor.matmul(
                    p[0:D, ts(k, NT)], what_k, x_aug[:, ts(j, NT)],
                    start=True, stop=True,
                )
            cols = len(grp) * NT
            nc.vector.tensor_copy(
                out=k_sb[:, grp[0] * NT : grp[0] * NT + cols], in_=p[0:D, 0:cols]
            )
        for grp in ((0, 1, 2), (3,)):
            p = ps_s.tile([MT, GRP * NT], F32, tag="sps")
            for k, j in enumerate(grp):
                nc.tensor.matmul(
                    p[0:D, ts(k, NT)], what_q, x_aug[:, ts(j, NT)],
                    start=True, stop=True,
                )
            cols = len(grp) * NT
            nc.vector.tensor_copy(
                out=q_sb[:, grp[0] * NT : grp[0] * NT + cols], in_=p[0:D, 0:cols]
            )
        # V'^T: 7 tiles of 65 columns fit in one PSUM bank (455 <= 512)
        m = 0
        while m < NM:
            cnt = min(7, NM - m)
            p = ps_s.tile([MT, GRP * NT], F32, tag="sps")
            for k in range(cnt):
                nc.tensor.matmul(
                    p[:, k * (D + 1) : (k + 1) * (D + 1)],
                    x_aug[:, ts(m + k, MT)],
                    what_v,
                    start=True, stop=True,
                )
            nc.vector.tensor_copy(
                out=vt_sb[:, m * (D + 1) : (m + cnt) * (D + 1)],
                in_=p[:, 0 : cnt * (D + 1)],
            )
            m += cnt

        # ---- attention main loop ---------------------------------------
        groups = [tuple(range(g, min(g + GRP, NM))) for g in range(0, NM, GRP)]
        for j in range(NJ):
            o_ps = ps_o.tile([D + 1, NT], F32, tag="o")
            q_j = q_sb[:, ts(j, NT)]
            for grp in groups:
                s_ps = ps_s.tile([MT, GRP * NT], F32, tag="sps")
                for k, mm in enumerate(grp):
                    nc.tensor.matmul(
                        s_ps[:, ts(k, NT)], k_sb[:, ts(mm, MT)], q_j,
                        start=True, stop=True,
                    )
                cols = len(grp) * NT
                e_sb = exps.tile([MT, GRP * NT], F32, tag="e")
                nc.scalar.activation(
                    out=e_sb[:, 0:cols],
                    in_=s_ps[:, 0:cols],
                    func=mybir.ActivationFunctionType.Exp,
                    scale=float(SCALE),
                )
                for k, mm in enumerate(grp):
                    nc.tensor.matmul(
                        o_ps,
                        vt_sb[:, mm * (D + 1) : (mm + 1) * (D + 1)],
                        e_sb[:, ts(k, NT)],
                        start=(mm == 0),
                        stop=(mm == NM - 1),
                    )

            # ---- epilogue: project + normalize + store -----------------
            o_sb = outp.tile([D + 1, NT], F32, tag="osb")
            nc.vector.tensor_copy(out=o_sb, in_=o_ps)
            rec = outp.tile([1, NT], F32, tag="rec")
            nc.vector.reciprocal(out=rec, in_=o_sb[D : D + 1, :])
            rb_ps = ps_m.tile([MT, NT], F32, tag="m")
            nc.tensor.matmul(rb_ps[0:D, :], ones_col, rec, start=True, stop=True)
            rb_sb = outp.tile([D, NT], F32, tag="rb")
            nc.vector.tensor_copy(out=rb_sb, in_=rb_ps[0:D, :])
            z_ps = ps_m.tile([MT, NT], F32, tag="m")
            nc.tensor.matmul(z_ps[0:D, :], woTa, o_sb, start=True, stop=True)
            y_sb = outp.tile([D, NT], F32, tag="y")
            nc.vector.tensor_mul(out=y_sb, in0=z_ps[0:D, :], in1=rb_sb)
            dma.dma_start(out=out_d[:, ts(j, NT)], in_=y_sb)

    return nc


def _get_nc():
    if "nc" not in _cache:
        _cache["nc"] = _build_nc()
    return _cache["nc"]


def kernel(x, gn_weight, gn_bias, wq, bq, wk, bk, wv, bv, wo, bo):
    f = lambda a: np.ascontiguousarray(np.asarray(a, dtype=np.float32))
    x = f(x)
    shared = {
        "wqT": f(wq).T, "wkT": f(wk).T, "wvT": f(wv).T,
        "woTa": np.concatenate([f(wo).T, f(bo)[None, :]], axis=0),
        "bq_row": f(bq)[None, :], "bk_row": f(bk)[None, :],
        "bv_row": f(bv)[None, :],
        "gnw": f(gn_weight)[:, None], "gnb": f(gn_bias)[:, None],
    }
    shared = {k: np.ascontiguousarray(v) for k, v in shared.items()}
    in_maps = []
    for i in range(NCORES):
        b, h = divmod(i, 2)
        xb = x[b].reshape(C, N)
        if h:
            xb = np.concatenate([xb[:, NH:], xb[:, :NH]], axis=1)
        in_maps.append({"x": np.ascontiguousarray(xb), **shared})

    res = run_bass_kernel_spmd(_get_nc(), in_maps, core_ids=list(range(NCORES)))

    out = np.empty((B, D, N), dtype=np.float32)
    for i in range(NCORES):
        b, h = divmod(i, 2)
        out[b, :, h * NH : (h + 1) * NH] = res.results[i]["out"]
    return out.reshape(B, D, H, W)


# revision 6
# speedup vs baseline: 4.0676x; 4.0676x over previous
"""Bass/Trainium2 kernel for nn_AttentionHead (B=4, C=D=64, H=W=64).

Sharding: 8 cores = 4 batches x 2 query-halves. Each core holds the full
x for its batch (keys/values need all 4096 positions) and computes
attention for 2048 query positions. Per-core inputs are column-rotated so
every core's query block is columns [0, 2048) of its own x — the program
is identical across cores (SPMD), only data differs. Softmax over keys is
permutation-invariant, so rotating the key order is exact.

On-device math (per core, N=4096 keys, NH=2048 queries):
  GroupNorm(num_groups=C) is affine per (batch, channel): xn = s*x + t with
  s = gn_w*rsqrt(var+eps), t = gn_b - mu*s. Folded into the projections:
  Q = (wq*s) @ x + (wq@t + bq), etc. An all-ones row appended to x makes
  every projection bias a plain matmul contraction (x_aug is [65, 4096]).
  Scores are computed transposed, S^T[m,n] = K[:,m]^T Q[:,n], so softmax'
  numerator exp(S^T/8) lands with keys on partitions — ready to be the
  moving operand of the AV matmul. No max-subtraction (|s/8| < ~2.5 here,
  exp is safe in fp32). The softmax denominator comes from an extra
  all-ones column in V' (so O'[64,n] = sum_m exp), and the final division
  is folded past the output projection: y = (wo@O + bo*denom) * (1/denom),
  with bo riding as row 64 of the augmented wo^T.
"""

import sys

sys.path.insert(0, "/opt/trn_rl_repo")

from contextlib import ExitStack

import numpy as np

import concourse.bass as bass
import concourse.tile as tile
from concourse import mybir
from concourse.bass import ts
from concourse.bass_utils import run_bass_kernel_spmd

# ---------------------------------------------------------------------------
# Workaround: this toolchain's walrus accepts at most ONE semaphore wait per
# instruction, but Tile's scheduler (and its tail drain) can attach several.
# Post-process every block, hoisting excess waits onto InstEventSemaphore
# carriers inserted immediately before the offending instruction on the same
# engine (engines execute their streams in order => semantically identical).
from concourse.vector_clock import ScopedClock as _ScopedClock
from concourse.bass import _bass_rust as _br


def _split_multiwait_instructions(nc, h0):
    cur_bb = nc.cur_bb.bb
    for f in nc.m.functions:
        for bb in f.blocks:
            insts = list(bb.instructions)
            out = []
            changed = False
            for ins in insts:
                si = ins.sync_info
                if si is not None:
                    waits = list(si.on_wait)
                    if len(waits) > 1:
                        for w in waits[:-1]:
                            carrier = nc.engines[ins.engine].wait_ge(h0, 0).ins
                            lst = list(cur_bb.instructions)
                            assert lst and lst[-1].name == carrier.name
                            lst.pop()
                            cur_bb.instructions = lst
                            carrier.sync_info.on_wait = [w]
                            out.append(carrier)
                        si.on_wait = [waits[-1]]
                        changed = True
                out.append(ins)
            if changed:
                bb.instructions = out


def _patched_drain_and_barrier(self, tick_clock, wait_clock):
    nc = self.nc
    assert self.sems is not None
    h0 = next(iter(self.sems.allocated().values()), None)
    if h0 is not None:
        _split_multiwait_instructions(nc, h0)

    drain_inst = nc.sync.drain()
    wait_clock.add_sem_waits(
        drain_inst.ins, _ScopedClock({None: tick_clock.global_clock})
    )
    si = drain_inst.ins.sync_info
    if si is not None:
        waits = list(si.on_wait)
        if len(waits) > 1:
            si.on_wait = [waits[0]]
            for w in waits[1:]:
                d2 = nc.sync.drain()
                _br.wait_op(d2.ins, h0, 0, "sem-ge", False)
                d2.ins.sync_info.on_wait = [w]

    nc.all_engine_barrier()
    popped = nc._tile_sem_poison_stack.pop()
    assert popped is self._sem_poison
    nc.clear_and_free_semaphores(list(self.sems.allocated().values()))
    nc.all_engine_barrier()


tile.TileContext._drain_and_barrier = _patched_drain_and_barrier
# ---------------------------------------------------------------------------

B, C, D, H, W = 4, 64, 64, 64, 64
N = H * W  # 4096 spatial positions (keys)
NCORES = 8
NH = N // 2  # 2048 queries per core
NT = 512  # query-tile width (one PSUM bank)
MT = 128  # key-tile height (matmul partition dim)
NJ = NH // NT  # 4 query tiles
NM = N // MT  # 32 key tiles
GRP = 3  # key tiles per scores/exp group (3 PSUM banks)
EPS = 1e-5
SCALE = 1.0 / np.sqrt(np.float32(D))  # folded into exp's free affine
F32 = mybir.dt.float32

_cache = {}


def _build_nc():
    nc = bass.Bass()
    x_d = nc.declare_dram_parameter("x", [C, N], F32, isOutput=False)
    wqT_d = nc.declare_dram_parameter("wqT", [C, D], F32, isOutput=False)
    wkT_d = nc.declare_dram_parameter("wkT", [C, D], F32, isOutput=False)
    wvT_d = nc.declare_dram_parameter("wvT", [C, D], F32, isOutput=False)
    woTa_d = nc.declare_dram_parameter("woTa", [D + 1, D], F32, isOutput=False)
    bq_d = nc.declare_dram_parameter("bq_row", [1, D], F32, isOutput=False)
    bk_d = nc.declare_dram_parameter("bk_row", [1, D], F32, isOutput=False)
    bv_d = nc.declare_dram_parameter("bv_row", [1, D], F32, isOutput=False)
    gnw_d = nc.declare_dram_parameter("gnw", [C, 1], F32, isOutput=False)
    gnb_d = nc.declare_dram_parameter("gnb", [C, 1], F32, isOutput=False)
    out_d = nc.declare_dram_parameter("out", [D, NH], F32, isOutput=True)

    with tile.TileContext(nc) as tc, ExitStack() as ctx:
        consts = ctx.enter_context(tc.tile_pool(name="consts", bufs=1))
        big = ctx.enter_context(tc.tile_pool(name="big", bufs=1))
        exps = ctx.enter_context(tc.tile_pool(name="exps", bufs=2))
        outp = ctx.enter_context(tc.tile_pool(name="outp", bufs=2))
        ps_s = ctx.enter_context(tc.tile_pool(name="ps_s", bufs=2, space="PSUM"))
        ps_o = ctx.enter_context(tc.tile_pool(name="ps_o", bufs=1, space="PSUM"))
        ps_m = ctx.enter_context(tc.tile_pool(name="ps_m", bufs=1, space="PSUM"))

        # ---- load weights / params -------------------------------------
        wqT = consts.tile([C, D], F32)
        wkT = consts.tile([C, D], F32)
        wvT = consts.tile([C, D], F32)
        woTa = consts.tile([D + 1, D], F32)
        bq_row = consts.tile([1, D], F32)
        bk_row = consts.tile([1, D], F32)
        bv_row = consts.tile([1, D], F32)
        gnw = consts.tile([C, 1], F32)
        gnb = consts.tile([C, 1], F32)
        dma = nc.default_dma_engine
        dma.dma_start(out=wqT, in_=wqT_d[:])
        dma.dma_start(out=wkT, in_=wkT_d[:])
        dma.dma_start(out=wvT, in_=wvT_d[:])
        dma.dma_start(out=woTa, in_=woTa_d[:])
        dma.dma_start(out=bq_row, in_=bq_d[:])
        dma.dma_start(out=bk_row, in_=bk_d[:])
        dma.dma_start(out=bv_row, in_=bv_d[:])
        dma.dma_start(out=gnw, in_=gnw_d[:])
        dma.dma_start(out=gnb, in_=gnb_d[:])

        ones_col = consts.tile([1, D], F32)
        nc.gpsimd.memset(ones_col, 1.0)

        # ---- x with an all-ones row 64 ---------------------------------
        x_aug = big.tile([C + 1, N], F32)
        dma.dma_start(out=x_aug[0:C, :], in_=x_d[:])
        nc.gpsimd.memset(x_aug[C : C + 1, :], 1.0)

        # ---- GroupNorm stats: mean/var per channel over all 4096 -------
        stats = consts.tile([C, N // 512, 6], F32)
        for j in range(N // 512):
            nc.vector.bn_stats(out=stats[:, j, :], in_=x_aug[0:C, ts(j, 512)])
        mv = consts.tile([C, 2], F32)
        nc.vector.bn_aggr(out=mv, in_=stats)

        # rs = (var+eps)^-0.5 via Ln/Exp (keeps everything in one ACT
        # table set, natural_log_exp_and_others, shared with the main exp)
        vpe = consts.tile([C, 1], F32)
        epst = consts.tile([C, 1], F32)
        nc.gpsimd.memset(epst, EPS)
        nc.vector.tensor_add(out=vpe, in0=mv[:, 1:2], in1=epst)
        lnv = consts.tile([C, 1], F32)
        nc.scalar.activation(out=lnv, in_=vpe, func=mybir.ActivationFunctionType.Ln)
        rs = consts.tile([C, 1], F32)
        nc.scalar.activation(
            out=rs, in_=lnv, func=mybir.ActivationFunctionType.Exp, scale=-0.5
        )
        s_vec = consts.tile([C, 1], F32)
        nc.vector.tensor_mul(out=s_vec, in0=rs, in1=gnw)
        mus = consts.tile([C, 1], F32)
        nc.vector.tensor_mul(out=mus, in0=mv[:, 0:1], in1=s_vec)
        t_vec = consts.tile([C, 1], F32)
        nc.vector.tensor_sub(out=t_vec, in0=gnb, in1=mus)

        # ---- augmented projection weights ------------------------------
        # what_* rows 0..63 = w^T * s (per-channel), row 64 = (w@t + b)^T
        what_q = consts.tile([C + 1, D], F32)
        what_k = consts.tile([C + 1, D], F32)
        what_v = consts.tile([C + 1, D + 1], F32)
        nc.gpsimd.memset(what_v[:, D : D + 1], 0.0)
        nc.vector.tensor_scalar_mul(out=what_q[0:C, :], in0=wqT, scalar1=s_vec)
        nc.vector.tensor_scalar_mul(out=what_k[0:C, :], in0=wkT, scalar1=s_vec)
        nc.vector.tensor_scalar_mul(out=what_v[0:C, 0:D], in0=wvT, scalar1=s_vec)
        for whT, wT, b_row, col in (
            (what_q, wqT, bq_row, D),
            (what_k, wkT, bk_row, D),
            (what_v, wvT, bv_row, D),
        ):
            r_ps = ps_m.tile([MT, NT], F32, tag="m")
            nc.tensor.matmul(r_ps[0:1, 0:D], t_vec, wT, start=True, stop=True)
            nc.vector.tensor_add(
                out=whT[C : C + 1, 0:col], in0=r_ps[0:1, 0:D], in1=b_row
            )
        nc.gpsimd.memset(what_v[C : C + 1, D : D + 1], 1.0)

        # ---- projections ----------------------------------------------
        # K [64, 4096] and Q [64, 2048] channel-major; V'^T as 32 tiles of
        # [128 keys, 65] (64 channels + ones column), packed contiguously.
        k_sb = big.tile([D, N], F32)
        q_sb = big.tile([D, NH], F32)
        vt_sb = big.tile([MT, NM * (D + 1)], F32)

        for grp in ((0, 1, 2), (3, 4, 5), (6, 7)):
            p = ps_s.tile([MT, GRP * NT], F32, tag="sps")
            for k, j in enumerate(grp):
                nc.tensor.matmul(
                    p[0:D, ts(k, NT)], what_k, x_aug[:, ts(j, NT)],
                    start=True, stop=True,
                )
            cols = len(grp) * NT
            nc.vector.tensor_copy(
                out=k_sb[:, grp[0] * NT : grp[0] * NT + cols], in_=p[0:D, 0:cols]
            )
        for grp in ((0, 1, 2), (3,)):
            p = ps_s.tile([MT, GRP * NT], F32, tag="sps")
            for k, j in enumerate(grp):
                nc.tensor.matmul(
                    p[0:D, ts(k, NT)], what_q, x_aug[:, ts(j, NT)],
                    start=True, stop=True,
                )
            cols = len(grp) * NT
            nc.vector.tensor_copy(
                out=q_sb[:, grp[0] * NT : grp[0] * NT + cols], in_=p[0:D, 0:cols]
            )
        # V'^T: 7 tiles of 65 columns fit in one PSUM bank (455 <= 512)
        m = 0
        while m < NM:
            cnt = min(7, NM - m)
            p = ps_s.tile([MT, GRP * NT], F32, tag="sps")
            for k in range(cnt):
                nc.tensor.matmul(
                    p[:, k * (D + 1) : (k + 1) * (D + 1)],
                    x_aug[:, ts(m + k, MT)],
                    what_v,
                    start=True, stop=True,
                )
            nc.vector.tensor_copy(
                out=vt_sb[:, m * (D + 1) : (m + cnt) * (D + 1)],
                in_=p[:, 0 : cnt * (D + 1)],
            )
            m += cnt

        # ---- attention main loop ---------------------------------------
        groups = [tuple(range(g, min(g + GRP, NM))) for g in range(0, NM, GRP)]
        for j in range(NJ):
            o_ps = ps_o.tile([D + 1, NT], F32, tag="o")
            q_j = q_sb[:, ts(j, NT)]
            for grp in groups:
                s_ps = ps_s.tile([MT, GRP * NT], F32, tag="sps")
                for k, mm in enumerate(grp):
                    nc.tensor.matmul(
                        s_ps[:, ts(k, NT)], k_sb[:, ts(mm, MT)], q_j,
                        start=True, stop=True,
                    )
                cols = len(grp) * NT
                e_sb = exps.tile([MT, GRP * NT], F32, tag="e")
                nc.scalar.activation(
                    out=e_sb[:, 0:cols],
                    in_=s_ps[:, 0:cols],
                    func=mybir.ActivationFunctionType.Exp,
                    scale=float(SCALE),
                )
                for k, mm in enumerate(grp):
                    nc.tensor.matmul(
                        o_ps,
                        vt_sb[:, mm * (D + 1) : (mm + 1) * (D + 1)],
                        e_sb[:, ts(k, NT)],
                        start=(mm == 0),
                        stop=(mm == NM - 1),
                    )

            # ---- epilogue: project + normalize + store -----------------
            o_sb = outp.tile([D + 1, NT], F32, tag="osb")
            nc.vector.tensor_copy(out=o_sb, in_=o_ps)
            rec = outp.tile([1, NT], F32, tag="rec")
            nc.vector.reciprocal(out=rec, in_=o_sb[D : D + 1, :])
            rb_ps = ps_m.tile([MT, NT], F32, tag="m")
            nc.tensor.matmul(rb_ps[0:D, :], ones_col, rec, start=True, stop=True)
            rb_sb = outp.tile([D, NT], F32, tag="rb")
            nc.vector.tensor_copy(out=rb_sb, in_=rb_ps[0:D, :])
            z_ps = ps_m.tile([MT, NT], F32, tag="m")
            nc.tensor.matmul(z_ps[0:D, :], woTa, o_sb, start=True, stop=True)
            y_sb = outp.tile([D, NT], F32, tag="y")
            nc.vector.tensor_mul(out=y_sb, in0=z_ps[0:D, :], in1=rb_sb)
            dma.dma_start(out=out_d[:, ts(j, NT)], in_=y_sb)

    return nc


def _get_nc():
    if "nc" not in _cache:
        _cache["nc"] = _build_nc()
    return _cache["nc"]


class _Runner:
    """Cached SPMD executor: builds the shard_map'd jit once so repeat calls
    skip retracing (run_bass_via_pjrt rebuilds its jit on every call)."""

    def __init__(self, nc, n_cores):
        import jax
        from jax.sharding import Mesh, PartitionSpec
        from jax.experimental.shard_map import shard_map
        from concourse import bass2jax
        from concourse import mybir as _mb

        bass2jax.install_neuronx_cc_hook()
        partition_name = (
            nc.partition_id_tensor.name if nc.partition_id_tensor else None
        )
        in_names, out_names, out_avals, zero_outs = [], [], [], []
        for alloc in nc.m.functions[0].allocations:
            if not isinstance(alloc, _mb.MemoryLocationSet):
                continue
            name = alloc.memorylocations[0].name
            if alloc.kind == "ExternalInput":
                if name != partition_name:
                    in_names.append(name)
            elif alloc.kind == "ExternalOutput":
                out_names.append(name)
                shape = tuple(alloc.tensor_shape)
                dtype = _mb.dt.np(alloc.dtype)
                out_avals.append(jax.core.ShapedArray(shape, dtype))
                zero_outs.append(np.zeros(shape, dtype))
        self.in_names = list(in_names)
        self.out_names = list(out_names)
        self.out_avals = out_avals
        self.zero_outs = zero_outs
        n_params = len(in_names)
        all_in_names = in_names + out_names
        if partition_name is not None:
            all_in_names = all_in_names + [partition_name]

        def _body(*args):
            operands = list(args)
            if partition_name is not None:
                operands.append(bass2jax.partition_id_tensor())
            outs = bass2jax._bass_exec_p.bind(
                *operands,
                out_avals=tuple(out_avals),
                in_names=tuple(all_in_names),
                out_names=tuple(out_names),
                lowering_input_output_aliases=(),
                sim_require_finite=True,
                sim_require_nnan=True,
                nc=nc,
            )
            return tuple(outs)

        devices = jax.devices()[:n_cores]
        mesh = Mesh(np.asarray(devices), ("core",))
        n_outs = len(out_names)
        self.n_cores = n_cores
        self.fn = jax.jit(
            shard_map(
                _body,
                mesh=mesh,
                in_specs=(PartitionSpec("core"),) * (n_params + n_outs),
                out_specs=(PartitionSpec("core"),) * n_outs,
                check_rep=False,
            ),
            keep_unused=True,
        )

    def concat_inputs(self, in_maps):
        cat = [
            np.concatenate([m[name] for m in in_maps], axis=0)
            for name in self.in_names
        ]
        cat += [
            np.zeros((self.n_cores * z.shape[0], *z.shape[1:]), z.dtype)
            for z in self.zero_outs
        ]
        return cat

    def __call__(self, concat_in):
        return self.fn(*concat_in)

    def run(self, in_maps):
        import jax

        out_arrs = jax.block_until_ready(self(self.concat_inputs(in_maps)))
        return [
            {
                name: np.asarray(out_arrs[i]).reshape(
                    self.n_cores, *self.out_avals[i].shape
                )[c]
                for i, name in enumerate(self.out_names)
            }
            for c in range(self.n_cores)
        ]


def _get_runner():
    if "runner" not in _cache:
        _cache["runner"] = _Runner(_get_nc(), NCORES)
    return _cache["runner"]


def _make_in_maps(x, gn_weight, gn_bias, wq, bq, wk, bk, wv, bv, wo, bo):
    f = lambda a: np.ascontiguousarray(np.asarray(a, dtype=np.float32))
    x = f(x)
    shared = {
        "wqT": f(wq).T, "wkT": f(wk).T, "wvT": f(wv).T,
        "woTa": np.concatenate([f(wo).T, f(bo)[None, :]], axis=0),
        "bq_row": f(bq)[None, :], "bk_row": f(bk)[None, :],
        "bv_row": f(bv)[None, :],
        "gnw": f(gn_weight)[:, None], "gnb": f(gn_bias)[:, None],
    }
    shared = {k: np.ascontiguousarray(v) for k, v in shared.items()}
    in_maps = []
    for i in range(NCORES):
        b, h = divmod(i, 2)
        xb = x[b].reshape(C, N)
        if h:
            xb = np.concatenate([xb[:, NH:], xb[:, :NH]], axis=1)
        in_maps.append({"x": np.ascontiguousarray(xb), **shared})
    return in_maps


def kernel(x, gn_weight, gn_bias, wq, bq, wk, bk, wv, bv, wo, bo):
    in_maps = _make_in_maps(x, gn_weight, gn_bias, wq, bq, wk, bk, wv, bv, wo, bo)
    results = _get_runner().run(in_maps)
    out = np.empty((B, D, N), dtype=np.float32)
    for i in range(NCORES):
        b, h = divmod(i, 2)
        out[b, :, h * NH : (h + 1) * NH] = results[i]["out"]
    return out.reshape(B, D, H, W)
